# revision 8
# baseline (speedup 1.0000x reference)
"""Trainium2 Bass kernel for nn_Attn_32925219291574.

Math: reference computes softmax_s( v . (W @ [hidden; enc[b,s]] + b) ).
Split W = [Wh | We]. The hidden/bias part v.(Wh@hidden + b) is constant in s,
and softmax is shift-invariant, so the output is exactly
    softmax_s( enc[b,s,:] . u ),   u = v @ We    (We = W[:, H:2H])
`hidden` and `b` never affect the output. The kernel streams the 256 MiB
encoder_outputs tensor once (memory-bound), computing per-row dot products
with a fused DVE multiply+row-sum (scalar_tensor_tensor + accum_out), then
does a 2D softmax per batch.

Sharding: data-parallel over batch B=16 -> 2 batches per core. We (4 MB) is
column-sharded 8 ways; each core computes its 128 entries of u on the PE and
an AllGather assembles the full u — saves 3.5 MB of HBM traffic per core.
"""

import numpy as np
from contextlib import ExitStack

import concourse.bass as bass
import concourse.bacc as bacc
import concourse.tile as tile
from concourse import mybir
from concourse.masks import make_identity
from concourse.bass_utils import run_bass_kernel_spmd

# Problem shapes (hardcoded per contest contract)
B, S, H = 16, 4096, 1024
NCORES = 8
B_LOC = B // NCORES            # 2 batches per core
ROWS = B_LOC * S               # 8192 rows of enc per core
P = 128
N_TILES = ROWS // P            # 64 tiles of [128, 1024]
TILES_PER_CHUNK = 4            # DMA chunk = [128, 4, 1024] = 2 MiB
N_CHUNKS = N_TILES // TILES_PER_CHUNK   # 16
KC = H // P                    # 8 contraction chunks for u = v @ We
TILES_PER_BATCH = S // P       # 32 score columns per batch
CHUNKS_PER_BATCH = N_CHUNKS // B_LOC    # 8

F32 = mybir.dt.float32

# set by test.py to capture a profile; harness leaves these untouched
TRACE = False
TMPDIR = None
LAST_RESULT = None


def _softmax_batch(nc, b, scores, smalls, psum_sm, identity, ones_row, ones_col,
                   out_ap):
    """Softmax over one batch's [128, 32] score block + store to HBM."""
    sb = scores[:, b * TILES_PER_BATCH : (b + 1) * TILES_PER_BATCH]
    # global max: per-partition max -> transpose -> max -> -M
    m1 = smalls.tile([P, 1], F32, tag=f"m1_{b}")
    nc.vector.tensor_reduce(out=m1, in_=sb, axis=mybir.AxisListType.X,
                            op=mybir.AluOpType.max)
    p_m1T = psum_sm.tile([1, P], F32, tag="pm1T")
    nc.tensor.transpose(p_m1T, m1, identity)
    negM = smalls.tile([1, 1], F32, tag=f"negM_{b}")
    nc.vector.tensor_reduce(out=negM, in_=p_m1T, axis=mybir.AxisListType.X,
                            op=mybir.AluOpType.max, negate=True)
    # broadcast -M to [128,1] via ones_row.T @ negM
    p_negMb = psum_sm.tile([P, 1], F32, tag="pnegMb")
    nc.tensor.matmul(p_negMb, lhsT=ones_row, rhs=negM, start=True, stop=True)
    negMb = smalls.tile([P, 1], F32, tag=f"negMb_{b}")
    nc.vector.tensor_copy(out=negMb, in_=p_negMb)
    # P = exp(scores - M) with free per-partition row sums
    pexp = smalls.tile([P, TILES_PER_BATCH], F32, tag=f"pexp_{b}")
    s1 = smalls.tile([P, 1], F32, tag=f"s1_{b}")
    nc.scalar.activation(out=pexp, in_=sb,
                         func=mybir.ActivationFunctionType.Exp,
                         bias=negMb, scale=1.0, accum_out=s1)
    # total sum across partitions: s1.T @ ones_col -> [1,1]
    p_S = psum_sm.tile([1, 1], F32, tag="pS")
    nc.tensor.matmul(p_S, lhsT=s1, rhs=ones_col, start=True, stop=True)
    r_S = smalls.tile([1, 1], F32, tag=f"rS_{b}")
    nc.vector.reciprocal(out=r_S, in_=p_S)
    p_rb = psum_sm.tile([P, 1], F32, tag="prb")
    nc.tensor.matmul(p_rb, lhsT=ones_row, rhs=r_S, start=True, stop=True)
    rb = smalls.tile([P, 1], F32, tag=f"rb_{b}")
    nc.vector.tensor_copy(out=rb, in_=p_rb)
    y = smalls.tile([P, TILES_PER_BATCH], F32, tag=f"y_{b}")
    nc.vector.tensor_scalar_mul(out=y, in0=pexp, scalar1=rb)
    # transpose [128, 32] -> [32, 128] so the HBM store is contiguous
    p_yt = psum_sm.tile([TILES_PER_BATCH, P], F32, tag="pyt")
    nc.tensor.transpose(p_yt, y, identity)
    yt = smalls.tile([TILES_PER_BATCH, P], F32, tag=f"yt_{b}")
    nc.vector.tensor_copy(out=yt, in_=p_yt)
    nc.sync.dma_start(out=out_ap[b, 0, :].rearrange("(t p) -> t p", p=P), in_=yt)


def _emit(ctx: ExitStack, tc: tile.TileContext, enc_h, we_h, v_h, out_h,
          cc_in, cc_out):
    nc = tc.nc
    enc_ap = enc_h[:, :, :]
    we_ap = we_h[:, :]
    v_ap = v_h[:, :]
    out_ap = out_h[:, :, :]

    singles = ctx.enter_context(tc.tile_pool(name="singles", bufs=1))
    chunks = ctx.enter_context(tc.tile_pool(name="chunks", bufs=5))
    smalls = ctx.enter_context(tc.tile_pool(name="smalls", bufs=1))
    psum_u_pool = ctx.enter_context(tc.tile_pool(name="psum_u", bufs=1, space="PSUM"))
    psum_sm = ctx.enter_context(tc.tile_pool(name="psum_sm", bufs=1, space="PSUM"))

    # constants used by softmax (emit early; they are off the critical path)
    identity = singles.tile([P, P], F32)
    make_identity(nc, identity)
    ones_row = singles.tile([1, P], F32)
    nc.vector.memset(ones_row, 1.0)
    ones_col = singles.tile([P, 1], F32)
    nc.vector.memset(ones_col, 1.0)

    # ---- phase 0: u_loc[j] = sum_k v[k] * We_loc[k, j] on the PE ----------
    # We_loc is this core's [H, 128] column slice; rows (k) go on partitions.
    we_sb = singles.tile([P, KC, P], F32)
    nc.sync.dma_start(out=we_sb, in_=we_ap.rearrange("(kc p) j -> p kc j", p=P))
    v_sb = singles.tile([P, KC], F32)
    nc.sync.dma_start(out=v_sb, in_=v_ap[0, :].rearrange("(kc p) -> p kc", p=P))

    psum_ul = psum_u_pool.tile([P, 1], F32, tag="ul")
    for kc in range(KC):
        nc.tensor.matmul(
            psum_ul,
            lhsT=we_sb[:, kc, :],
            rhs=v_sb[:, kc : kc + 1],
            start=(kc == 0),
            stop=(kc == KC - 1),
        )
    u_col = singles.tile([P, 1], F32)
    nc.vector.tensor_copy(out=u_col, in_=psum_ul)

    # ---- AllGather the 8 x 128 u chunks into the full u -------------------
    nc.sync.dma_start(out=cc_in[:], in_=u_col)
    nc.gpsimd.collective_compute(
        "AllGather", mybir.AluOpType.bypass,
        replica_groups=[list(range(NCORES))],
        ins=[cc_in[:]], outs=[cc_out[:]],
    )
    u_row = singles.tile([1, H], F32)
    nc.sync.dma_start(out=u_row, in_=cc_out[:].rearrange("(o f) -> o f", o=1))

    # broadcast u to all 128 partitions: ones_row.T @ u_row
    psum_ub = psum_u_pool.tile([P, H], F32, tag="ub")
    for nh in range(2):
        nc.tensor.matmul(
            psum_ub[:, nh * 512 : (nh + 1) * 512],
            lhsT=ones_row,
            rhs=u_row[0:1, nh * 512 : (nh + 1) * 512],
            start=True,
            stop=True,
        )
    u_bcast = singles.tile([P, H], F32)
    nc.vector.tensor_copy(out=u_bcast, in_=psum_ub)

    # ---- main loop: scores[r] = enc_row[r] . u ----------------------------
    scores = singles.tile([P, N_TILES], F32)   # col c*4+t, row p -> flat row
    scratch = singles.tile([P, H], F32)        # STT mandatory full-product dump
    enc_r = enc_ap.flatten_outer_dims().rearrange(
        "(c t p) h -> c p t h", p=P, t=TILES_PER_CHUNK
    )
    for c in range(N_CHUNKS):
        ch = chunks.tile([P, TILES_PER_CHUNK, H], F32)
        nc.sync.dma_start(out=ch, in_=enc_r[c])
        for t in range(TILES_PER_CHUNK):
            col = c * TILES_PER_CHUNK + t
            # fused multiply+row-sum on DVE via standard TensorScalarPtr:
            # out = (in0 * 1.0) * in1, accum_out = sum(out)
            nc.vector.scalar_tensor_tensor(
                out=scratch,
                in0=ch[:, t, :],
                scalar=1.0,
                in1=u_bcast,
                op0=mybir.AluOpType.mult,
                op1=mybir.AluOpType.mult,
                accum_out=scores[:, col : col + 1],
            )
        # softmax for a batch as soon as its 32 score columns are done
        if c == CHUNKS_PER_BATCH - 1:
            _softmax_batch(nc, 0, scores, smalls, psum_sm, identity, ones_row,
                           ones_col, out_ap)
        elif c == N_CHUNKS - 1:
            _softmax_batch(nc, 1, scores, smalls, psum_sm, identity, ones_row,
                           ones_col, out_ap)


def build_bass():
    nc = bacc.Bacc("TRN2", target_bir_lowering=False)
    enc_h = nc.dram_tensor("enc", [B_LOC, S, H], F32, kind="ExternalInput")
    we_h = nc.dram_tensor("we", [H, P], F32, kind="ExternalInput")
    v_h = nc.dram_tensor("v", [1, H], F32, kind="ExternalInput")
    out_h = nc.dram_tensor("out", [B_LOC, 1, S], F32, kind="ExternalOutput")
    cc_in = nc.dram_tensor("cc_in", [P], F32)
    cc_out = nc.dram_tensor("cc_out", [H], F32, addr_space="Shared")
    with ExitStack() as ctx:
        tc = ctx.enter_context(tile.TileContext(nc))
        _emit(ctx, tc, enc_h, we_h, v_h, out_h, cc_in, cc_out)
    nc.compile()
    return nc


_NC = None


def _get_nc():
    global _NC
    if _NC is None:
        _NC = build_bass()
    return _NC


def kernel(hidden, encoder_outputs, W, b, v):
    global LAST_RESULT
    nc = _get_nc()
    W = np.asarray(W, dtype=np.float32)
    v2 = np.ascontiguousarray(np.asarray(v, dtype=np.float32))
    enc = np.asarray(encoder_outputs, dtype=np.float32)
    in_maps = [
        {
            "enc": np.ascontiguousarray(enc[i * B_LOC : (i + 1) * B_LOC]),
            # core i computes u[i*128:(i+1)*128] from its We column slice
            "we": np.ascontiguousarray(W[:, H + i * P : H + (i + 1) * P]),
            "v": v2,
        }
        for i in range(NCORES)
    ]
    res = run_bass_kernel_spmd(nc, in_maps, core_ids=list(range(NCORES)),
                               trace=TRACE, tmpdir=TMPDIR)
    LAST_RESULT = res
    return np.concatenate([res.results[i]["out"] for i in range(NCORES)], axis=0)


# revision 11
# speedup vs baseline: 1.3508x; 1.3508x over previous
"""Trainium2 Bass kernel for nn_Attn_32925219291574.

Math: reference computes softmax_s( v . (W @ [hidden; enc[b,s]] + b) ).
Split W = [Wh | We]. The hidden/bias part v.(Wh@hidden + b) is constant in s,
and softmax is shift-invariant, so the output is exactly
    softmax_s( enc[b,s,:] . u ),   u = v @ We    (We = W[:, H:2H])
`hidden` and `b` never affect the output. The kernel streams the 256 MiB
encoder_outputs tensor once (memory-bound), computing per-row dot products
with a fused DVE multiply+row-sum (scalar_tensor_tensor + accum_out), then
does a 2D softmax per batch.

Sharding: data-parallel over batch B=16 -> 2 batches per core, no cross-core
communication. We (4 MB) loads first (enc DMAs are held behind it) so the
u = v @ We bootstrap finishes early; u is reduced on DVE as We chunks land,
summed across partitions on the PE, and broadcast to all 128 partitions.
"""

import numpy as np
from contextlib import ExitStack

import concourse.bass as bass
import concourse.bacc as bacc
import concourse.tile as tile
from concourse import mybir
from concourse.masks import make_identity
from concourse.tile_rust import add_dep_helper
from concourse.bass_utils import run_bass_kernel_spmd

# Problem shapes (hardcoded per contest contract)
B, S, H = 16, 4096, 1024
NCORES = 8
B_LOC = B // NCORES            # 2 batches per core
ROWS = B_LOC * S               # 8192 rows of enc per core
P = 128
N_TILES = ROWS // P            # 64 tiles of [128, 1024]
TILES_PER_CHUNK = 4            # DMA chunk = [128, 4, 1024] = 2 MiB
N_CHUNKS = N_TILES // TILES_PER_CHUNK   # 16
KC = H // P                    # 8 contraction chunks for u = v @ We
TILES_PER_BATCH = S // P       # 32 score columns per batch
CHUNKS_PER_BATCH = N_CHUNKS // B_LOC    # 8
ENC_BUFS = 5

F32 = mybir.dt.float32

# set by test.py to capture a profile; harness leaves these untouched
TRACE = False
TMPDIR = None
LAST_RESULT = None


def _softmax_batch(nc, b, scores, smalls, psum_sm, identity, ones_row, ones_col,
                   out_ap):
    """Softmax over one batch's [128, 32] score block + store to HBM."""
    sb = scores[:, b * TILES_PER_BATCH : (b + 1) * TILES_PER_BATCH]
    # global max: per-partition max -> transpose -> max -> -M
    m1 = smalls.tile([P, 1], F32, tag=f"m1_{b}")
    nc.vector.tensor_reduce(out=m1, in_=sb, axis=mybir.AxisListType.X,
                            op=mybir.AluOpType.max)
    p_m1T = psum_sm.tile([1, P], F32, tag="sm")
    nc.tensor.transpose(p_m1T, m1, identity)
    negM = smalls.tile([1, 1], F32, tag=f"negM_{b}")
    nc.vector.tensor_reduce(out=negM, in_=p_m1T, axis=mybir.AxisListType.X,
                            op=mybir.AluOpType.max, negate=True)
    # broadcast -M to [128,1] via ones_row.T @ negM
    p_negMb = psum_sm.tile([P, 1], F32, tag="sm")
    nc.tensor.matmul(p_negMb, lhsT=ones_row, rhs=negM, start=True, stop=True)
    negMb = smalls.tile([P, 1], F32, tag=f"negMb_{b}")
    nc.vector.tensor_copy(out=negMb, in_=p_negMb)
    # P = exp(scores - M) with free per-partition row sums
    pexp = smalls.tile([P, TILES_PER_BATCH], F32, tag=f"pexp_{b}")
    s1 = smalls.tile([P, 1], F32, tag=f"s1_{b}")
    nc.scalar.activation(out=pexp, in_=sb,
                         func=mybir.ActivationFunctionType.Exp,
                         bias=negMb, scale=1.0, accum_out=s1)
    # total sum across partitions: s1.T @ ones_col -> [1,1]
    p_S = psum_sm.tile([1, 1], F32, tag="sm")
    nc.tensor.matmul(p_S, lhsT=s1, rhs=ones_col, start=True, stop=True)
    r_S = smalls.tile([1, 1], F32, tag=f"rS_{b}")
    nc.vector.reciprocal(out=r_S, in_=p_S)
    p_rb = psum_sm.tile([P, 1], F32, tag="sm")
    nc.tensor.matmul(p_rb, lhsT=ones_row, rhs=r_S, start=True, stop=True)
    rb = smalls.tile([P, 1], F32, tag=f"rb_{b}")
    nc.vector.tensor_copy(out=rb, in_=p_rb)
    y = smalls.tile([P, TILES_PER_BATCH], F32, tag=f"y_{b}")
    nc.vector.tensor_scalar_mul(out=y, in0=pexp, scalar1=rb)
    # transpose [128, 32] -> [32, 128] so the HBM store is contiguous
    p_yt = psum_sm.tile([TILES_PER_BATCH, P], F32, tag="sm")
    nc.tensor.transpose(p_yt, y, identity)
    yt = smalls.tile([TILES_PER_BATCH, P], F32, tag=f"yt_{b}")
    nc.vector.tensor_copy(out=yt, in_=p_yt)
    nc.sync.dma_start(out=out_ap[b, 0, :].rearrange("(t p) -> t p", p=P), in_=yt)


def _emit(ctx: ExitStack, tc: tile.TileContext, enc_h, we_h, v_h, out_h):
    nc = tc.nc
    enc_ap = enc_h[:, :, :]
    we_ap = we_h[:, :]
    v_ap = v_h[:, :]
    out_ap = out_h[:, :, :]

    singles = ctx.enter_context(tc.tile_pool(name="singles", bufs=1))
    chunks = ctx.enter_context(tc.tile_pool(name="chunks", bufs=ENC_BUFS))
    smalls = ctx.enter_context(tc.tile_pool(name="smalls", bufs=1))
    psum_u_pool = ctx.enter_context(tc.tile_pool(name="psum_u", bufs=1, space="PSUM"))
    psum_sm = ctx.enter_context(tc.tile_pool(name="psum_sm", bufs=1, space="PSUM"))

    # constants used by softmax (cheap engine ops, off the critical path)
    identity = singles.tile([P, P], F32)
    make_identity(nc, identity)
    ones_row = singles.tile([1, P], F32)
    nc.vector.memset(ones_row, 1.0)
    ones_col = singles.tile([P, 1], F32)
    nc.vector.memset(ones_col, 1.0)

    # ---- phase 0: load We first (8 x 512 KB chunks), v alongside ----------
    v_sb = singles.tile([P, KC], F32)
    nc.sync.dma_start(out=v_sb, in_=v_ap[0, :].rearrange("(kc p) -> p kc", p=P))
    we_sb = singles.tile([P, KC, H], F32)
    we_r = we_ap.rearrange("(kc p) h -> kc p h", p=P)
    we_dmas = []
    for kc in range(KC):
        di = nc.sync.dma_start(out=we_sb[:, kc, :], in_=we_r[kc])
        we_dmas.append(di.ins)

    # acc_kc = We_kc * v_kc + acc_{kc-1}, one DVE op per chunk (pipelines
    # with the chunk DMAs); ping-pong buffers to avoid in-place aliasing
    acc_a = singles.tile([P, H], F32)
    acc_b = singles.tile([P, H], F32)
    accs = [acc_a, acc_b]
    nc.vector.tensor_scalar_mul(out=acc_a, in0=we_sb[:, 0, :],
                                scalar1=v_sb[:, 0:1])
    for kc in range(1, KC):
        nc.vector.scalar_tensor_tensor(
            out=accs[kc % 2], in0=we_sb[:, kc, :], scalar=v_sb[:, kc : kc + 1],
            in1=accs[(kc + 1) % 2],
            op0=mybir.AluOpType.mult, op1=mybir.AluOpType.add)
    acc = accs[(KC - 1) % 2]

    # partition-sum each 128-column slice of acc on the PE: u as 8 columns
    psum_uc = psum_u_pool.tile([P, KC], F32, tag="uc")
    for jc in range(KC):
        nc.tensor.matmul(psum_uc[:, jc : jc + 1],
                         lhsT=acc[:, jc * P : (jc + 1) * P],
                         rhs=ones_col, start=True, stop=True)
    uc = singles.tile([P, KC], F32)
    nc.vector.tensor_copy(out=uc, in_=psum_uc)
    # transpose each u chunk column to a [1, 128] row -> u_row = [1, 1024]
    p_urow = psum_u_pool.tile([1, H], F32, tag="urow")
    for jc in range(KC):
        nc.tensor.transpose(p_urow[0:1, jc * P : (jc + 1) * P],
                            uc[:, jc : jc + 1], identity)
    u_row = singles.tile([1, H], F32)
    nc.vector.tensor_copy(out=u_row, in_=p_urow)
    # broadcast to all 128 partitions: ones_row.T @ u_row
    psum_ub = psum_u_pool.tile([P, H], F32, tag="ub")
    for nh in range(2):
        nc.tensor.matmul(psum_ub[:, nh * 512 : (nh + 1) * 512],
                         lhsT=ones_row, rhs=u_row[0:1, nh * 512 : (nh + 1) * 512],
                         start=True, stop=True)
    u_bcast = singles.tile([P, H], F32)
    nc.vector.tensor_copy(out=u_bcast, in_=psum_ub)

    # ---- main loop: scores[r] = enc_row[r] . u ----------------------------
    scores = singles.tile([P, N_TILES], F32)   # col c*4+t, row p -> flat row
    scratch = singles.tile([P, H], F32)        # STT mandatory full-product dump
    enc_r = enc_ap.flatten_outer_dims().rearrange(
        "(c t p) h -> c p t h", p=P, t=TILES_PER_CHUNK
    )
    for c in range(N_CHUNKS):
        ch = chunks.tile([P, TILES_PER_CHUNK, H], F32)
        di = nc.sync.dma_start(out=ch, in_=enc_r[c])
        if c < ENC_BUFS:
            # hold early enc DMAs behind the We load so the u bootstrap
            # gets full HBM bandwidth (SDMA round-robins queues otherwise)
            add_dep_helper(di.ins, we_dmas[-1], sync=True,
                           reason="prioritize We load over enc prefetch")
        for t in range(TILES_PER_CHUNK):
            col = c * TILES_PER_CHUNK + t
            # fused multiply+row-sum on DVE via standard TensorScalarPtr:
            # out = (in0 * 1.0) * in1, accum_out = sum(out)
            nc.vector.scalar_tensor_tensor(
                out=scratch,
                in0=ch[:, t, :],
                scalar=1.0,
                in1=u_bcast,
                op0=mybir.AluOpType.mult,
                op1=mybir.AluOpType.mult,
                accum_out=scores[:, col : col + 1],
            )
        # softmax for a batch as soon as its 32 score columns are done
        if c == CHUNKS_PER_BATCH - 1:
            _softmax_batch(nc, 0, scores, smalls, psum_sm, identity, ones_row,
                           ones_col, out_ap)
        elif c == N_CHUNKS - 1:
            _softmax_batch(nc, 1, scores, smalls, psum_sm, identity, ones_row,
                           ones_col, out_ap)


def build_bass():
    nc = bacc.Bacc("TRN2", target_bir_lowering=False)
    enc_h = nc.dram_tensor("enc", [B_LOC, S, H], F32, kind="ExternalInput")
    we_h = nc.dram_tensor("we", [H, H], F32, kind="ExternalInput")
    v_h = nc.dram_tensor("v", [1, H], F32, kind="ExternalInput")
    out_h = nc.dram_tensor("out", [B_LOC, 1, S], F32, kind="ExternalOutput")
    with ExitStack() as ctx:
        tc = ctx.enter_context(tile.TileContext(nc))
        _emit(ctx, tc, enc_h, we_h, v_h, out_h)
    nc.compile()
    return nc


_NC = None


def _get_nc():
    global _NC
    if _NC is None:
        _NC = build_bass()
    return _NC


def kernel(hidden, encoder_outputs, W, b, v):
    global LAST_RESULT
    nc = _get_nc()
    we = np.ascontiguousarray(np.asarray(W, dtype=np.float32)[:, H:])
    v2 = np.ascontiguousarray(np.asarray(v, dtype=np.float32))
    enc = np.asarray(encoder_outputs, dtype=np.float32)
    in_maps = [
        {
            "enc": np.ascontiguousarray(enc[i * B_LOC : (i + 1) * B_LOC]),
            "we": we,
            "v": v2,
        }
        for i in range(NCORES)
    ]
    res = run_bass_kernel_spmd(nc, in_maps, core_ids=list(range(NCORES)),
                               trace=TRACE, tmpdir=TMPDIR)
    LAST_RESULT = res
    return np.concatenate([res.results[i]["out"] for i in range(NCORES)], axis=0)


# revision 14
# speedup vs baseline: 1.3734x; 1.0167x over previous
"""Trainium2 Bass kernel for nn_Attn_32925219291574.

Math: reference computes softmax_s( v . (W @ [hidden; enc[b,s]] + b) ).
Split W = [Wh | We]. The hidden/bias part v.(Wh@hidden + b) is constant in s,
and softmax is shift-invariant, so the output is exactly
    softmax_s( enc[b,s,:] . u ),   u = v @ We    (We = W[:, H:2H])
`hidden` and `b` never affect the output. The kernel streams the 256 MiB
encoder_outputs tensor once (memory-bound), computing per-row dot products
with a fused DVE multiply+row-sum (scalar_tensor_tensor + accum_out), then
does a 2D softmax per batch.

Sharding: data-parallel over batch B=16 -> 2 batches per core, no cross-core
communication. We (4 MB) loads first (enc DMAs are held behind it) so the
u = v @ We bootstrap finishes early; u is reduced on DVE as We chunks land,
summed across partitions on the PE, and broadcast to all 128 partitions.
"""

import numpy as np
from contextlib import ExitStack

import concourse.bass as bass
import concourse.bacc as bacc
import concourse.tile as tile
from concourse import mybir
from concourse.masks import make_identity
from concourse.tile_rust import add_dep_helper
from concourse.bass_utils import run_bass_kernel_spmd

# Problem shapes (hardcoded per contest contract)
B, S, H = 16, 4096, 1024
NCORES = 8
B_LOC = B // NCORES            # 2 batches per core
ROWS = B_LOC * S               # 8192 rows of enc per core
P = 128
N_TILES = ROWS // P            # 64 tiles of [128, 1024]
TILES_PER_CHUNK = 4            # DMA chunk = [128, 4, 1024] = 2 MiB
N_CHUNKS = N_TILES // TILES_PER_CHUNK   # 16
KC = H // P                    # 8 contraction chunks for u = v @ We
TILES_PER_BATCH = S // P       # 32 score columns per batch
CHUNKS_PER_BATCH = N_CHUNKS // B_LOC    # 8
ENC_BUFS = 7

F32 = mybir.dt.float32

# set by test.py to capture a profile; harness leaves these untouched
TRACE = False
TMPDIR = None
LAST_RESULT = None


def _softmax_batch(nc, b, scores, smalls, psum_sm, identity, ones_row, ones_col,
                   out_ap):
    """Softmax over one batch's [128, 32] score block + store to HBM."""
    sb = scores[:, b * TILES_PER_BATCH : (b + 1) * TILES_PER_BATCH]
    # global max: per-partition max -> transpose -> max -> -M
    m1 = smalls.tile([P, 1], F32, tag=f"m1_{b}")
    nc.vector.tensor_reduce(out=m1, in_=sb, axis=mybir.AxisListType.X,
                            op=mybir.AluOpType.max)
    p_m1T = psum_sm.tile([1, P], F32, tag="sm")
    nc.tensor.transpose(p_m1T, m1, identity)
    negM = smalls.tile([1, 1], F32, tag=f"negM_{b}")
    nc.vector.tensor_reduce(out=negM, in_=p_m1T, axis=mybir.AxisListType.X,
                            op=mybir.AluOpType.max, negate=True)
    # broadcast -M to [128,1] via ones_row.T @ negM
    p_negMb = psum_sm.tile([P, 1], F32, tag="sm")
    nc.tensor.matmul(p_negMb, lhsT=ones_row, rhs=negM, start=True, stop=True)
    negMb = smalls.tile([P, 1], F32, tag=f"negMb_{b}")
    nc.vector.tensor_copy(out=negMb, in_=p_negMb)
    # P = exp(scores - M) with free per-partition row sums
    pexp = smalls.tile([P, TILES_PER_BATCH], F32, tag=f"pexp_{b}")
    s1 = smalls.tile([P, 1], F32, tag=f"s1_{b}")
    nc.scalar.activation(out=pexp, in_=sb,
                         func=mybir.ActivationFunctionType.Exp,
                         bias=negMb, scale=1.0, accum_out=s1)
    # total sum across partitions: s1.T @ ones_col -> [1,1]
    p_S = psum_sm.tile([1, 1], F32, tag="sm")
    nc.tensor.matmul(p_S, lhsT=s1, rhs=ones_col, start=True, stop=True)
    r_S = smalls.tile([1, 1], F32, tag=f"rS_{b}")
    nc.vector.reciprocal(out=r_S, in_=p_S)
    p_rb = psum_sm.tile([P, 1], F32, tag="sm")
    nc.tensor.matmul(p_rb, lhsT=ones_row, rhs=r_S, start=True, stop=True)
    rb = smalls.tile([P, 1], F32, tag=f"rb_{b}")
    nc.vector.tensor_copy(out=rb, in_=p_rb)
    y = smalls.tile([P, TILES_PER_BATCH], F32, tag=f"y_{b}")
    nc.vector.tensor_scalar_mul(out=y, in0=pexp, scalar1=rb)
    # transpose [128, 32] -> [32, 128] so the HBM store is contiguous
    p_yt = psum_sm.tile([TILES_PER_BATCH, P], F32, tag="sm")
    nc.tensor.transpose(p_yt, y, identity)
    yt = smalls.tile([TILES_PER_BATCH, P], F32, tag=f"yt_{b}")
    nc.vector.tensor_copy(out=yt, in_=p_yt)
    nc.sync.dma_start(out=out_ap[b, 0, :].rearrange("(t p) -> t p", p=P), in_=yt)


def _emit(ctx: ExitStack, tc: tile.TileContext, enc_h, we_h, v_h, out_h):
    nc = tc.nc
    enc_ap = enc_h[:, :, :]
    we_ap = we_h[:, :]
    v_ap = v_h[:, :]
    out_ap = out_h[:, :, :]

    singles = ctx.enter_context(tc.tile_pool(name="singles", bufs=1))
    chunks = ctx.enter_context(tc.tile_pool(name="chunks", bufs=ENC_BUFS))
    smalls = ctx.enter_context(tc.tile_pool(name="smalls", bufs=1))
    psum_u_pool = ctx.enter_context(tc.tile_pool(name="psum_u", bufs=1, space="PSUM"))
    psum_sm = ctx.enter_context(tc.tile_pool(name="psum_sm", bufs=1, space="PSUM"))

    # constants used by softmax (cheap engine ops, off the critical path)
    identity = singles.tile([P, P], F32)
    make_identity(nc, identity)
    ones_row = singles.tile([1, P], F32)
    nc.vector.memset(ones_row, 1.0)
    ones_col = singles.tile([P, 1], F32)
    nc.vector.memset(ones_col, 1.0)

    # ---- phase 0: load We first (8 x 512 KB chunks), v alongside ----------
    v_sb = singles.tile([P, KC], F32)
    nc.sync.dma_start(out=v_sb, in_=v_ap[0, :].rearrange("(kc p) -> p kc", p=P))
    we_sb = singles.tile([P, KC, H], F32)
    we_r = we_ap.rearrange("(kc p) h -> kc p h", p=P)
    we_dmas = []
    for kc in range(KC):
        di = nc.sync.dma_start(out=we_sb[:, kc, :], in_=we_r[kc])
        we_dmas.append(di.ins)

    # acc_kc = We_kc * v_kc + acc_{kc-1}, one DVE op per chunk (pipelines
    # with the chunk DMAs); ping-pong buffers to avoid in-place aliasing
    acc_a = singles.tile([P, H], F32)
    acc_b = singles.tile([P, H], F32)
    accs = [acc_a, acc_b]
    nc.vector.tensor_scalar_mul(out=acc_a, in0=we_sb[:, 0, :],
                                scalar1=v_sb[:, 0:1])
    for kc in range(1, KC):
        nc.vector.scalar_tensor_tensor(
            out=accs[kc % 2], in0=we_sb[:, kc, :], scalar=v_sb[:, kc : kc + 1],
            in1=accs[(kc + 1) % 2],
            op0=mybir.AluOpType.mult, op1=mybir.AluOpType.add)
    acc = accs[(KC - 1) % 2]

    # warm the PE (HAM clock gate) right before the u matmuls: junk
    # transposes that depend on the last We chunk so they run just-in-time
    p_junk = psum_u_pool.tile([P, P], F32, tag="junk")
    for _ in range(24):
        nc.tensor.transpose(p_junk, we_sb[:, KC - 1, 0:P], identity)

    # partition-sum each 128-column slice of acc on the PE, directly as a
    # row: u_row[0, jc*128+n] = sum_k acc[k, jc*128+n]
    p_urow = psum_u_pool.tile([1, H], F32, tag="urow")
    for jc in range(KC):
        nc.tensor.matmul(p_urow[0:1, jc * P : (jc + 1) * P],
                         lhsT=ones_col, rhs=acc[:, jc * P : (jc + 1) * P],
                         start=True, stop=True)
    u_row = singles.tile([1, H], F32)
    nc.vector.tensor_copy(out=u_row, in_=p_urow)
    # broadcast to all 128 partitions: ones_row.T @ u_row
    psum_ub = psum_u_pool.tile([P, H], F32, tag="ub")
    for nh in range(2):
        nc.tensor.matmul(psum_ub[:, nh * 512 : (nh + 1) * 512],
                         lhsT=ones_row, rhs=u_row[0:1, nh * 512 : (nh + 1) * 512],
                         start=True, stop=True)
    u_bcast = singles.tile([P, H], F32)
    nc.vector.tensor_copy(out=u_bcast, in_=psum_ub)

    # ---- main loop: scores[r] = enc_row[r] . u ----------------------------
    scores = singles.tile([P, N_TILES], F32)   # col, row p -> flat row col*128+p
    scratch = singles.tile([P, H], F32)        # STT mandatory full-product dump
    enc_flat = enc_ap.flatten_outer_dims()     # [8192, 1024]
    # 15 x 4-tile chunks + 2 x 2-tile chunks: the small final chunks shorten
    # the last-chunk -> last-STT -> softmax tail
    chunk_sizes = [TILES_PER_CHUNK] * 15 + [2, 2]
    col0 = 0
    for c, nt in enumerate(chunk_sizes):
        ch = chunks.tile([P, TILES_PER_CHUNK, H], F32, tag="ch")
        src = enc_flat[col0 * P : (col0 + nt) * P, :].rearrange(
            "(t p) h -> p t h", p=P)
        di = nc.sync.dma_start(out=ch[:, 0:nt, :], in_=src)
        if c < ENC_BUFS:
            # hold early enc DMAs behind the We load so the u bootstrap
            # gets full HBM bandwidth (SDMA round-robins queues otherwise)
            add_dep_helper(di.ins, we_dmas[-1], sync=True,
                           reason="prioritize We load over enc prefetch")
        for t in range(nt):
            col = col0 + t
            # fused multiply+row-sum on DVE via standard TensorScalarPtr:
            # out = (in0 * 1.0) * in1, accum_out = sum(out)
            nc.vector.scalar_tensor_tensor(
                out=scratch,
                in0=ch[:, t, :],
                scalar=1.0,
                in1=u_bcast,
                op0=mybir.AluOpType.mult,
                op1=mybir.AluOpType.mult,
                accum_out=scores[:, col : col + 1],
            )
        col0 += nt
        # softmax for a batch as soon as its 32 score columns are done
        if col0 == TILES_PER_BATCH:
            _softmax_batch(nc, 0, scores, smalls, psum_sm, identity, ones_row,
                           ones_col, out_ap)
        elif col0 == N_TILES:
            _softmax_batch(nc, 1, scores, smalls, psum_sm, identity, ones_row,
                           ones_col, out_ap)


def build_bass():
    nc = bacc.Bacc("TRN2", target_bir_lowering=False)
    enc_h = nc.dram_tensor("enc", [B_LOC, S, H], F32, kind="ExternalInput")
    we_h = nc.dram_tensor("we", [H, H], F32, kind="ExternalInput")
    v_h = nc.dram_tensor("v", [1, H], F32, kind="ExternalInput")
    out_h = nc.dram_tensor("out", [B_LOC, 1, S], F32, kind="ExternalOutput")
    with ExitStack() as ctx:
        tc = ctx.enter_context(tile.TileContext(nc))
        _emit(ctx, tc, enc_h, we_h, v_h, out_h)
    nc.compile()
    return nc


_NC = None


def _get_nc():
    global _NC
    if _NC is None:
        _NC = build_bass()
    return _NC


def kernel(hidden, encoder_outputs, W, b, v):
    global LAST_RESULT
    nc = _get_nc()
    we = np.ascontiguousarray(np.asarray(W, dtype=np.float32)[:, H:])
    v2 = np.ascontiguousarray(np.asarray(v, dtype=np.float32))
    enc = np.asarray(encoder_outputs, dtype=np.float32)
    in_maps = [
        {
            "enc": np.ascontiguousarray(enc[i * B_LOC : (i + 1) * B_LOC]),
            "we": we,
            "v": v2,
        }
        for i in range(NCORES)
    ]
    res = run_bass_kernel_spmd(nc, in_maps, core_ids=list(range(NCORES)),
                               trace=TRACE, tmpdir=TMPDIR)
    LAST_RESULT = res
    return np.concatenate([res.results[i]["out"] for i in range(NCORES)], axis=0)
